# revision 34
# baseline (speedup 1.0000x reference)
"""Trainium2 Bass kernel for nn_ExLRestSelfAtten (sparse local attention).

Math notes (exact simplifications of the reference):
  - The reference softmaxes over a singleton axis, so atten_weights == 1.0
    exactly, and ctx[b,t] = sum_{j=0..4} vals[b,t,j].
  - ctx @ W_out = windowsum5(relu(x@W_in + b_in)) @ (W_v @ W_out), because the
    5-wide window sum is linear and commutes with the output projection.
  - W_q / W_k only feed the (constant) softmax: dead code for both outputs.

Device strategy (pure batch data-parallel, 2 batches per core):
  - Host pre-transposes x to [B, D, S] so each core streams xT tiles directly
    as the matmul moving operand (contraction dim D on partitions).
  - x tiles [128, 2(dc), 512] are loaded by 8 DMAs alternating between the
    two HWDGE rings (SP via nc.sync, ACT via nc.scalar) in the order PE
    consumes them; each dma_start costs ~650ns serial on its ring.
  - A block of dummy matmuls at kernel start warms the PE HAM clock gate
    during the DMA ramp.
  - Stage 1 (PE): hT[hc] = W_in[:,hc]^T @ xT, f32r matmuls (1 cyc/col).
  - ReLU + b_in fused into the PSUM->SBUF copy on the scalar engine.
  - Stage 2 (PE): y^T = Wc^T @ relu_hT, one exact-bank [2, 512] PSUM tile
    per 512-token group (f32r outputs must sit at PSUM partition 0, uniform
    N=512); DVE packs the 4 groups into [98, 516] SBUF at 32-aligned
    partitions with 2-token halos stitched from neighbor groups (zeros at
    batch edges), then one 5-wide window sum per batch.
  - Output: 2 partition-strided DMAs per batch ([4, 512] x 4 quadrants).
  - Host unpacks the (c, q) layout and adds b_out.
"""

import numpy as np

import concourse.bass as bass
import concourse.mybir as mybir
import concourse.tile as tile
from concourse.tile import ScopedClock
from concourse.bass_utils import run_bass_kernel_spmd

F32 = mybir.dt.float32
F32R = mybir.dt.float32r

B, S, D, H, DOUT = 16, 2048, 256, 256, 2
PAD = 2                       # window half-width
NCORES = 8
BPC = B // NCORES             # batches per core = 2
TB = 512                      # stage-1 moving-dim block
NTB = S // TB                 # 4
G = 512                       # stage-2 tokens per group (exact PSUM bank)
NG = S // G                   # 4 groups per batch
WAVE = 4
N_WARM = 8                    # PE warm-up matmuls


def _split_multi_waits(nc, max_waits=1):
    """This walrus build rejects >max_waits sem waits on a single
    instruction: hoist excess waits onto same-engine NoOps placed
    immediately before the instruction (engines execute in order)."""
    counter = [0]

    def wait_nop(engine, w):
        nop = mybir.InstNoOp(name=f"ant-waitsplit-{counter[0]}", ins=[], outs=[])
        counter[0] += 1
        nop.engine = engine
        nop.sync_info = mybir.SyncInfo(on_wait=[w], on_update=[])
        return nop

    for bb in nc.main_func.blocks:
        out = []
        changed = False
        for ins in bb.instructions:
            si = ins.sync_info
            waits = list(si.on_wait) if si is not None else []
            if len(waits) > max_waits:
                for w in waits[:-max_waits]:
                    out.append(wait_nop(ins.engine, w))
                si.on_wait = waits[-max_waits:]
                changed = True
            out.append(ins)
        if changed:
            bb.instructions = out


class _MinTailTileContext(tile.TileContext):
    """Tail: drain (split into 1-wait ops by the post-pass) + ONE barrier.
    Skips the semaphore clear + second barrier: this NEFF is executed
    once per compile, so sem state need not be restored."""

    def __init__(self, nc, minimal_tail=True, **kw):
        super().__init__(nc, **kw)
        self._minimal_tail = minimal_tail

    def _drain_and_barrier(self, tick_clock, wait_clock):
        if not self._minimal_tail:
            return super()._drain_and_barrier(tick_clock, wait_clock)
        drain_inst = self.nc.sync.drain()
        wait_clock.add_sem_waits(
            drain_inst.ins, ScopedClock({None: tick_clock.global_clock})
        )
        self.nc.all_engine_barrier()
        assert self.sems is not None
        popped = self.nc._tile_sem_poison_stack.pop()
        assert popped is self._sem_poison


def build_nc(use_f32r=True, minimal_tail=True):
    MDT = F32R if use_f32r else F32
    nc = bass.Bass()
    xt = nc.declare_dram_parameter("xt", [BPC, D, S], MDT, isOutput=False)
    w_in = nc.declare_dram_parameter("w_in", [D, H], MDT, isOutput=False)
    b_in = nc.declare_dram_parameter("b_in", [H, 1], F32, isOutput=False)
    wc = nc.declare_dram_parameter("wc", [H, DOUT], MDT, isOutput=False)
    y = nc.declare_dram_parameter("y", [BPC, 2, NG, G], F32, isOutput=True)

    rings = [nc.sync, nc.scalar]

    with _MinTailTileContext(nc, minimal_tail=minimal_tail) as tc:
        with (
            tc.tile_pool(name="consts", bufs=1) as consts,
            tc.tile_pool(name="xpool", bufs=2) as xpool,
            tc.tile_pool(name="hpool", bufs=2) as hpool,
            tc.tile_pool(name="wpool", bufs=2) as wpool,
            tc.tile_pool(name="opool", bufs=2) as opool,
            tc.tile_pool(name="psh", bufs=4, space="PSUM") as psh_pool,
            tc.tile_pool(name="psy", bufs=4, space="PSUM") as psy_pool,
        ):
            # PE warm-up: HAM un-throttles after ~3.4us of sustained PE
            # activity; burn that window on dummy matmuls while DMAs run.
            wz = consts.tile([128, 128], F32, tag="wz")
            nc.vector.memset(wz[:], 0.0)
            pwarm = psy_pool.tile([128, 128], F32, tag="psy", name="pwarm")
            for i in range(N_WARM):
                nc.tensor.matmul(
                    pwarm[:], wz[:], wz[:], start=(i == 0), stop=(i == N_WARM - 1)
                )

            # weights / bias first: small, so they clear the rings early
            wt = consts.tile([128, 2, H], MDT, tag="wt")       # [k, dc, h]
            nc.sync.dma_start(wt[:], w_in.rearrange("(dc k) h -> k dc h", dc=2))
            bt = consts.tile([128, 2], F32, tag="bt")          # [k, hc]
            nc.scalar.dma_start(bt[:], b_in.rearrange("(hc k) o -> k (hc o)", hc=2))
            wct = consts.tile([128, 2, DOUT], MDT, tag="wct")  # [k, hc, c]
            nc.scalar.dma_start(wct[:], wc.rearrange("(hc k) c -> k hc c", hc=2))

            # x loads: [128, 2(dc), TB] per (b, tb) in consumption order.
            # One shared tag with few bufs CASCADES the loads: concurrent
            # DMAs fair-share HBM, so capping in-flight transfers makes
            # early tiles complete early and PE can chase the stream.
            # (An explicit DMA->DMA dep chain measured the same mean but
            # intermittently wedged the device; slot-recycling is stable.)
            xb = {}
            for b in range(BPC):
                for tb in range(NTB):
                    t0 = tb * TB
                    xb[b, tb] = xpool.tile(
                        [128, 2, TB], MDT, tag="x", bufs=4, name=f"xb{b}{tb}"
                    )
                    rings[tb % 2].dma_start(
                        xb[b, tb][:],
                        xt[b, :, t0 : t0 + TB].rearrange(
                            "(dc p) t -> p dc t", p=128
                        ),
                    )

            for b in range(BPC):
                # relu(h)^T (batch edges handled by clipped stage-2 ranges)
                rh = {}
                for hc in range(2):
                    rh[hc] = hpool.tile(
                        [128, S], MDT, tag=f"rh{hc}", name=f"rh{hc}"
                    )

                # stage 1, latency order: tb outer so compute chases DMAs
                for tb in range(NTB):
                    for hc in range(2):
                        psh = psh_pool.tile(
                            [128, TB], F32, tag="psh", name=f"psh{hc}{tb}"
                        )
                        for dc in range(2):
                            nc.tensor.matmul(
                                psh[:],
                                wt[:, dc, hc * 128 : (hc + 1) * 128],
                                xb[b, tb][:, dc, :],
                                start=(dc == 0),
                                stop=(dc == 1),
                            )
                        if b == BPC - 1 and tb == NTB - 1:
                            for hh in range(2):
                                s = hh * (TB // 2)
                                nc.scalar.activation(
                                    rh[hc][:, tb * TB + s : tb * TB + s + TB // 2],
                                    psh[:, s : s + TB // 2],
                                    mybir.ActivationFunctionType.Relu,
                                    bias=bt[:, hc : hc + 1],
                                )
                        else:
                            nc.scalar.activation(
                                rh[hc][:, tb * TB : (tb + 1) * TB],
                                psh[:],
                                mybir.ActivationFunctionType.Relu,
                                bias=bt[:, hc : hc + 1],
                            )

                # stage 2: 4 groups of 512 tokens, each an exact-bank
                # [2, 512] PSUM tile at partition 0 (f32r), uniform N=512.
                psy = []
                for q in range(NG):
                    p = psy_pool.tile([2, G], F32, tag="psy", name=f"psy{q}")
                    if b == BPC - 1 and q == NG - 1:
                        # free-dim halves: start right after each relu half
                        for hh in range(2):
                            s = hh * (G // 2)
                            for hc in range(2):
                                nc.tensor.matmul(
                                    p[:, s : s + G // 2],
                                    wct[:, hc, :],
                                    rh[hc][:, q * G + s : q * G + s + G // 2],
                                    start=(hc == 0),
                                    stop=(hc == 1),
                                )
                    else:
                        for hc in range(2):
                            nc.tensor.matmul(
                                p[:],
                                wct[:, hc, :],
                                rh[hc][:, q * G : (q + 1) * G],
                                start=(hc == 0),
                                stop=(hc == 1),
                            )
                    psy.append(p)

                # pack all 4 groups into [98, 516] at 32-aligned partitions
                # with 2-token halos stitched from neighbor groups (zeros at
                # batch edges). For the LAST batch (kernel tail) split the
                # main copies across ACT and DVE — ACT is done with relu by
                # then, so the pack fills in parallel.
                pk = wpool.tile([98, G + 2 * PAD], F32, tag="pk")
                nc.vector.memset(pk[0:2, 0:PAD], 0.0)
                nc.vector.memset(pk[96:98, G + PAD :], 0.0)
                for q in range(NG):
                    r = pk[32 * q : 32 * q + 2, :]
                    if b == BPC - 1 and q % 2 == 1:
                        nc.scalar.activation(
                            r[:, PAD : G + PAD],
                            psy[q][:],
                            mybir.ActivationFunctionType.Copy,
                        )
                    else:
                        nc.vector.tensor_copy(r[:, PAD : G + PAD], psy[q][:])
                    if q > 0:
                        nc.vector.tensor_copy(
                            r[:, 0:PAD], psy[q - 1][:, G - PAD : G]
                        )
                    if q < NG - 1:
                        nc.vector.tensor_copy(
                            r[:, G + PAD :], psy[q + 1][:, 0:PAD]
                        )

                # 5-wide window sum in two free-dim halves so the first
                # half's output DMA overlaps the second half's adds.
                t1 = wpool.tile([98, G], F32, tag="t1")
                t2 = wpool.tile([98, G], F32, tag="t2")
                gy = wpool.tile([98, G], F32, tag="gy")
                HH = G // 2
                for h in range(2):
                    s = h * HH
                    nc.vector.tensor_add(
                        t1[:, s : s + HH], pk[:, s : s + HH], pk[:, s + 1 : s + 1 + HH]
                    )
                    nc.vector.tensor_add(
                        t2[:, s : s + HH], pk[:, s + 2 : s + 2 + HH], pk[:, s + 3 : s + 3 + HH]
                    )
                    nc.vector.tensor_add(
                        t1[:, s : s + HH], t1[:, s : s + HH], t2[:, s : s + HH]
                    )
                    nc.vector.tensor_add(
                        gy[:, s : s + HH], t1[:, s : s + HH], pk[:, s + 4 : s + 4 + HH]
                    )
                    # partition-strided output DMAs: c -> [4(q), HH]
                    for c in range(2):
                        rings[c].dma_start(
                            y[b, c, :, s : s + HH],
                            gy[c : c + 97 : 32, s : s + HH],
                            single_packet=True,
                        )

    _split_multi_waits(nc)
    return nc


_NC_CACHE = {}


def _get_nc(use_f32r=True, minimal_tail=True):
    key = (use_f32r, minimal_tail)
    if key not in _NC_CACHE:
        _NC_CACHE[key] = build_nc(use_f32r, minimal_tail)
    return _NC_CACHE[key]


def _run(x, W_in, b_in, W_v, W_out, b_out, use_f32r=True, minimal_tail=False,
         **spmd_kwargs):
    x = np.asarray(x, dtype=np.float32)
    W_in = np.ascontiguousarray(np.asarray(W_in, dtype=np.float32))
    b_in_col = np.ascontiguousarray(
        np.asarray(b_in, dtype=np.float32).reshape(H, 1)
    )
    Wc = (
        np.asarray(W_v, dtype=np.float64) @ np.asarray(W_out, dtype=np.float64)
    ).astype(np.float32)
    Wc = np.ascontiguousarray(Wc)

    # [B, S, D] -> [B, D, S] contiguous
    xT = np.ascontiguousarray(np.swapaxes(x, 1, 2))

    in_maps = [
        {
            "xt": xT[c * BPC : (c + 1) * BPC],
            "w_in": W_in,
            "b_in": b_in_col,
            "wc": Wc,
        }
        for c in range(NCORES)
    ]
    nc = _get_nc(use_f32r, minimal_tail)
    res = run_bass_kernel_spmd(
        nc, in_maps, core_ids=list(range(NCORES)), **spmd_kwargs
    )

    out = np.empty((B, S, DOUT), dtype=np.float32)
    for c in range(NCORES):
        yc = res.results[c]["y"]  # [BPC, 2(c), NG(q), G]
        for b in range(BPC):
            arr = yc[b].transpose(1, 2, 0).reshape(S, DOUT)  # t = q*G + i
            out[c * BPC + b] = arr
    out += np.asarray(b_out, dtype=np.float32).reshape(1, 1, DOUT)

    atten = np.ones((B, S, 1, 2 * PAD + 1), dtype=np.float32)
    return (out, atten), res


def kernel(x, W_in, b_in, W_q, W_k, W_v, W_out, b_out):
    (out, atten), _ = _run(x, W_in, b_in, W_v, W_out, b_out)
    return out, atten


# revision 36
# speedup vs baseline: 1.0281x; 1.0281x over previous
"""Trainium2 Bass kernel for nn_ExLRestSelfAtten (sparse local attention).

Math notes (exact simplifications of the reference):
  - The reference softmaxes over a singleton axis, so atten_weights == 1.0
    exactly, and ctx[b,t] = sum_{j=0..4} vals[b,t,j].
  - ctx @ W_out = windowsum5(relu(x@W_in + b_in)) @ (W_v @ W_out), because the
    5-wide window sum is linear and commutes with the output projection.
  - W_q / W_k only feed the (constant) softmax: dead code for both outputs.

Device strategy (pure batch data-parallel, 2 batches per core):
  - Host pre-transposes x to [B, D, S] so each core streams xT tiles directly
    as the matmul moving operand (contraction dim D on partitions).
  - x tiles [128, 2(dc), 512] are loaded by 8 DMAs alternating between the
    two HWDGE rings (SP via nc.sync, ACT via nc.scalar) in the order PE
    consumes them; each dma_start costs ~650ns serial on its ring.
  - A block of dummy matmuls at kernel start warms the PE HAM clock gate
    during the DMA ramp.
  - Stage 1 (PE): hT[hc] = W_in[:,hc]^T @ xT, f32r matmuls (1 cyc/col).
  - ReLU + b_in fused into the PSUM->SBUF copy on the scalar engine.
  - Stage 2 (PE): y^T = Wc^T @ relu_hT, one exact-bank [2, 512] PSUM tile
    per 512-token group (f32r outputs must sit at PSUM partition 0, uniform
    N=512); DVE packs the 4 groups into [98, 516] SBUF at 32-aligned
    partitions with 2-token halos stitched from neighbor groups (zeros at
    batch edges), then one 5-wide window sum per batch.
  - Output: 2 partition-strided DMAs per batch ([4, 512] x 4 quadrants).
  - Host unpacks the (c, q) layout and adds b_out.
"""

import numpy as np

import concourse.bass as bass
import concourse.mybir as mybir
import concourse.tile as tile
from concourse.tile import ScopedClock
from concourse.bass_utils import run_bass_kernel_spmd

F32 = mybir.dt.float32
F32R = mybir.dt.float32r

B, S, D, H, DOUT = 16, 2048, 256, 256, 2
PAD = 2                       # window half-width
NCORES = 8
BPC = B // NCORES             # batches per core = 2
TB = 512                      # stage-1 moving-dim block
NTB = S // TB                 # 4
G = 512                       # stage-2 tokens per group (exact PSUM bank)
NG = S // G                   # 4 groups per batch
WAVE = 4
N_WARM = 8                    # PE warm-up matmuls


def _split_multi_waits(nc, max_waits=1):
    """This walrus build rejects >max_waits sem waits on a single
    instruction: hoist excess waits onto same-engine NoOps placed
    immediately before the instruction (engines execute in order)."""
    counter = [0]

    def wait_nop(engine, w):
        nop = mybir.InstNoOp(name=f"ant-waitsplit-{counter[0]}", ins=[], outs=[])
        counter[0] += 1
        nop.engine = engine
        nop.sync_info = mybir.SyncInfo(on_wait=[w], on_update=[])
        return nop

    for bb in nc.main_func.blocks:
        out = []
        changed = False
        for ins in bb.instructions:
            si = ins.sync_info
            waits = list(si.on_wait) if si is not None else []
            if len(waits) > max_waits:
                for w in waits[:-max_waits]:
                    out.append(wait_nop(ins.engine, w))
                si.on_wait = waits[-max_waits:]
                changed = True
            out.append(ins)
        if changed:
            bb.instructions = out


class _MinTailTileContext(tile.TileContext):
    """Tail: drain (split into 1-wait ops by the post-pass) + ONE barrier.
    Skips the semaphore clear + second barrier: this NEFF is executed
    once per compile, so sem state need not be restored."""

    def __init__(self, nc, minimal_tail=True, **kw):
        super().__init__(nc, **kw)
        self._minimal_tail = minimal_tail

    def _drain_and_barrier(self, tick_clock, wait_clock):
        if not self._minimal_tail:
            return super()._drain_and_barrier(tick_clock, wait_clock)
        drain_inst = self.nc.sync.drain()
        wait_clock.add_sem_waits(
            drain_inst.ins, ScopedClock({None: tick_clock.global_clock})
        )
        self.nc.all_engine_barrier()
        assert self.sems is not None
        popped = self.nc._tile_sem_poison_stack.pop()
        assert popped is self._sem_poison


def build_nc(use_f32r=True, minimal_tail=True):
    MDT = F32R if use_f32r else F32
    nc = bass.Bass()
    xt = nc.declare_dram_parameter("xt", [BPC, D, S], MDT, isOutput=False)
    w_in = nc.declare_dram_parameter("w_in", [D, H], MDT, isOutput=False)
    b_in = nc.declare_dram_parameter("b_in", [H, 1], F32, isOutput=False)
    wc = nc.declare_dram_parameter("wc", [H, DOUT], MDT, isOutput=False)
    y = nc.declare_dram_parameter("y", [BPC, 2, NG, G], F32, isOutput=True)

    rings = [nc.sync, nc.scalar]

    with _MinTailTileContext(nc, minimal_tail=minimal_tail) as tc:
        with (
            tc.tile_pool(name="consts", bufs=1) as consts,
            tc.tile_pool(name="xpool", bufs=2) as xpool,
            tc.tile_pool(name="hpool", bufs=2) as hpool,
            tc.tile_pool(name="wpool", bufs=2) as wpool,
            tc.tile_pool(name="opool", bufs=2) as opool,
            tc.tile_pool(name="psh", bufs=4, space="PSUM") as psh_pool,
            tc.tile_pool(name="psy", bufs=4, space="PSUM") as psy_pool,
        ):
            # PE warm-up: HAM un-throttles after ~3.4us of sustained PE
            # activity; burn that window on dummy matmuls while DMAs run.
            wz = consts.tile([128, 128], F32, tag="wz")
            nc.vector.memset(wz[:], 0.0)
            pwarm = psy_pool.tile([128, 128], F32, tag="psy", name="pwarm")
            for i in range(N_WARM):
                nc.tensor.matmul(
                    pwarm[:], wz[:], wz[:], start=(i == 0), stop=(i == N_WARM - 1)
                )

            # weights / bias first: small, so they clear the rings early
            wt = consts.tile([128, 2, H], MDT, tag="wt")       # [k, dc, h]
            nc.sync.dma_start(wt[:], w_in.rearrange("(dc k) h -> k dc h", dc=2))
            bt = consts.tile([128, 2], F32, tag="bt")          # [k, hc]
            nc.scalar.dma_start(bt[:], b_in.rearrange("(hc k) o -> k (hc o)", hc=2))
            wct = consts.tile([128, 2, DOUT], MDT, tag="wct")  # [k, hc, c]
            nc.scalar.dma_start(wct[:], wc.rearrange("(hc k) c -> k hc c", hc=2))

            # x loads: [128, 2(dc), TB] per (b, tb) in consumption order.
            # One shared tag with few bufs CASCADES the loads: concurrent
            # DMAs fair-share HBM, so capping in-flight transfers makes
            # early tiles complete early and PE can chase the stream.
            # (An explicit DMA->DMA dep chain measured the same mean but
            # intermittently wedged the device; slot-recycling is stable.)
            xb = {}
            for b in range(BPC):
                for tb in range(NTB):
                    t0 = tb * TB
                    xb[b, tb] = xpool.tile(
                        [128, 2, TB], MDT, tag="x", bufs=4, name=f"xb{b}{tb}"
                    )
                    rings[tb % 2].dma_start(
                        xb[b, tb][:],
                        xt[b, :, t0 : t0 + TB].rearrange(
                            "(dc p) t -> p dc t", p=128
                        ),
                    )

            rh = {}

            def s1_tb(b, tb):
                for hc in range(2):
                    psh = psh_pool.tile(
                        [128, TB], F32, tag="psh", name=f"psh{b}{hc}{tb}"
                    )
                    for dc in range(2):
                        nc.tensor.matmul(
                            psh[:],
                            wt[:, dc, hc * 128 : (hc + 1) * 128],
                            xb[b, tb][:, dc, :],
                            start=(dc == 0),
                            stop=(dc == 1),
                        )
                    nc.scalar.activation(
                        rh[b, hc][:, tb * TB : (tb + 1) * TB],
                        psh[:],
                        mybir.ActivationFunctionType.Relu,
                        bias=bt[:, hc : hc + 1],
                    )

            def s2_group(b, q, psy):
                p = psy_pool.tile([2, G], F32, tag="psy", name=f"psy{b}{q}")
                for hc in range(2):
                    nc.tensor.matmul(
                        p[:],
                        wct[:, hc, :],
                        rh[b, hc][:, q * G : (q + 1) * G],
                        start=(hc == 0),
                        stop=(hc == 1),
                    )
                psy.append(p)

            def pack_window_out(b, psy):
                # pack groups into [98, 516] at 32-aligned partitions with
                # 2-token halos from neighbor groups (zeros at batch edges);
                # last batch splits main copies across ACT and DVE.
                pk = wpool.tile([98, G + 2 * PAD], F32, tag="pk", name=f"pk{b}")
                nc.vector.memset(pk[0:2, 0:PAD], 0.0)
                nc.vector.memset(pk[96:98, G + PAD :], 0.0)
                for q in range(NG):
                    r = pk[32 * q : 32 * q + 2, :]
                    if b == BPC - 1 and q % 2 == 1:
                        nc.scalar.activation(
                            r[:, PAD : G + PAD],
                            psy[q][:],
                            mybir.ActivationFunctionType.Copy,
                        )
                    else:
                        nc.vector.tensor_copy(r[:, PAD : G + PAD], psy[q][:])
                    if q > 0:
                        nc.vector.tensor_copy(
                            r[:, 0:PAD], psy[q - 1][:, G - PAD : G]
                        )
                    if q < NG - 1:
                        nc.vector.tensor_copy(
                            r[:, G + PAD :], psy[q + 1][:, 0:PAD]
                        )

                # 5-wide window sum in two free-dim halves; first half's
                # output DMA overlaps the second half's adds
                t1 = wpool.tile([98, G], F32, tag="t1", name=f"t1{b}")
                t2 = wpool.tile([98, G], F32, tag="t2", name=f"t2{b}")
                gy = wpool.tile([98, G], F32, tag="gy", name=f"gy{b}")
                HH = G // 2
                for h in range(2):
                    s = h * HH
                    nc.vector.tensor_add(
                        t1[:, s : s + HH], pk[:, s : s + HH], pk[:, s + 1 : s + 1 + HH]
                    )
                    nc.vector.tensor_add(
                        t2[:, s : s + HH], pk[:, s + 2 : s + 2 + HH], pk[:, s + 3 : s + 3 + HH]
                    )
                    nc.vector.tensor_add(
                        t1[:, s : s + HH], t1[:, s : s + HH], t2[:, s : s + HH]
                    )
                    nc.vector.tensor_add(
                        gy[:, s : s + HH], t1[:, s : s + HH], pk[:, s + 4 : s + 4 + HH]
                    )
                    for c in range(2):
                        rings[c].dma_start(
                            y[b, c, :, s : s + HH],
                            gy[c : c + 97 : 32, s : s + HH],
                        )

            for b in range(BPC):
                for hc in range(2):
                    rh[b, hc] = hpool.tile(
                        [128, S], MDT, tag=f"rh{hc}", name=f"rh{b}{hc}"
                    )

            # PE order: b0 stage-1; then b1 stage-1 tb-blocks INTERLEAVED
            # with b0 stage-2 groups (b0's relu deps are long satisfied, so
            # those MMs fill PE bubbles while b1 tiles arrive); then b1
            # stage-2. NG == NTB makes the interleave 1:1.
            psy0, psy1 = [], []
            for tb in range(NTB):
                s1_tb(0, tb)
            for tb in range(NTB):
                s1_tb(1, tb)
                s2_group(0, tb, psy0)
            pack_window_out(0, psy0)
            for q in range(NG):
                s2_group(1, q, psy1)
            pack_window_out(1, psy1)

    _split_multi_waits(nc)
    return nc


_NC_CACHE = {}


def _get_nc(use_f32r=True, minimal_tail=True):
    key = (use_f32r, minimal_tail)
    if key not in _NC_CACHE:
        _NC_CACHE[key] = build_nc(use_f32r, minimal_tail)
    return _NC_CACHE[key]


def _run(x, W_in, b_in, W_v, W_out, b_out, use_f32r=True, minimal_tail=False,
         **spmd_kwargs):
    x = np.asarray(x, dtype=np.float32)
    W_in = np.ascontiguousarray(np.asarray(W_in, dtype=np.float32))
    b_in_col = np.ascontiguousarray(
        np.asarray(b_in, dtype=np.float32).reshape(H, 1)
    )
    Wc = (
        np.asarray(W_v, dtype=np.float64) @ np.asarray(W_out, dtype=np.float64)
    ).astype(np.float32)
    Wc = np.ascontiguousarray(Wc)

    # [B, S, D] -> [B, D, S] contiguous
    xT = np.ascontiguousarray(np.swapaxes(x, 1, 2))

    in_maps = [
        {
            "xt": xT[c * BPC : (c + 1) * BPC],
            "w_in": W_in,
            "b_in": b_in_col,
            "wc": Wc,
        }
        for c in range(NCORES)
    ]
    nc = _get_nc(use_f32r, minimal_tail)
    res = run_bass_kernel_spmd(
        nc, in_maps, core_ids=list(range(NCORES)), **spmd_kwargs
    )

    out = np.empty((B, S, DOUT), dtype=np.float32)
    for c in range(NCORES):
        yc = res.results[c]["y"]  # [BPC, 2(c), NG(q), G]
        for b in range(BPC):
            arr = yc[b].transpose(1, 2, 0).reshape(S, DOUT)  # t = q*G + i
            out[c * BPC + b] = arr
    out += np.asarray(b_out, dtype=np.float32).reshape(1, 1, DOUT)

    atten = np.ones((B, S, 1, 2 * PAD + 1), dtype=np.float32)
    return (out, atten), res


def kernel(x, W_in, b_in, W_q, W_k, W_v, W_out, b_out):
    (out, atten), _ = _run(x, W_in, b_in, W_v, W_out, b_out)
    return out, atten


# revision 38
# speedup vs baseline: 1.0499x; 1.0211x over previous
"""Trainium2 Bass kernel for nn_ExLRestSelfAtten (sparse local attention).

Math notes (exact simplifications of the reference):
  - The reference softmaxes over a singleton axis, so atten_weights == 1.0
    exactly, and ctx[b,t] = sum_{j=0..4} vals[b,t,j].
  - ctx @ W_out = windowsum5(relu(x@W_in + b_in)) @ (W_v @ W_out), because the
    5-wide window sum is linear and commutes with the output projection.
  - W_q / W_k only feed the (constant) softmax: dead code for both outputs.

Device strategy (pure batch data-parallel, 2 batches per core):
  - Host pre-transposes x to [B, D, S] so each core streams xT tiles directly
    as the matmul moving operand (contraction dim D on partitions).
  - x tiles [128, 2(dc), 512] are loaded by 8 DMAs alternating between the
    two HWDGE rings (SP via nc.sync, ACT via nc.scalar) in the order PE
    consumes them; each dma_start costs ~650ns serial on its ring.
  - A block of dummy matmuls at kernel start warms the PE HAM clock gate
    during the DMA ramp.
  - Stage 1 (PE): hT[hc] = W_in[:,hc]^T @ xT, f32r matmuls (1 cyc/col).
  - ReLU + b_in fused into the PSUM->SBUF copy on the scalar engine.
  - Stage 2 (PE): y^T = Wc^T @ relu_hT, one exact-bank [2, 512] PSUM tile
    per 512-token group (f32r outputs must sit at PSUM partition 0, uniform
    N=512); DVE packs the 4 groups into [98, 516] SBUF at 32-aligned
    partitions with 2-token halos stitched from neighbor groups (zeros at
    batch edges), then one 5-wide window sum per batch.
  - Output: 2 partition-strided DMAs per batch ([4, 512] x 4 quadrants).
  - Host unpacks the (c, q) layout and adds b_out.
"""

import numpy as np

import concourse.bass as bass
import concourse.mybir as mybir
import concourse.tile as tile
from concourse.tile import ScopedClock
from concourse.bass_utils import run_bass_kernel_spmd

F32 = mybir.dt.float32
F32R = mybir.dt.float32r

B, S, D, H, DOUT = 16, 2048, 256, 256, 2
PAD = 2                       # window half-width
NCORES = 8
BPC = B // NCORES             # batches per core = 2
TB = 512                      # stage-1 moving-dim block
NTB = S // TB                 # 4
G = 512                       # stage-2 tokens per group (exact PSUM bank)
NG = S // G                   # 4 groups per batch
WAVE = 4
N_WARM = 8                    # PE warm-up matmuls


def _split_multi_waits(nc, max_waits=1):
    """This walrus build rejects >max_waits sem waits on a single
    instruction: hoist excess waits onto same-engine NoOps placed
    immediately before the instruction (engines execute in order)."""
    counter = [0]

    def wait_nop(engine, w):
        nop = mybir.InstNoOp(name=f"ant-waitsplit-{counter[0]}", ins=[], outs=[])
        counter[0] += 1
        nop.engine = engine
        nop.sync_info = mybir.SyncInfo(on_wait=[w], on_update=[])
        return nop

    for bb in nc.main_func.blocks:
        out = []
        changed = False
        for ins in bb.instructions:
            si = ins.sync_info
            waits = list(si.on_wait) if si is not None else []
            if len(waits) > max_waits:
                for w in waits[:-max_waits]:
                    out.append(wait_nop(ins.engine, w))
                si.on_wait = waits[-max_waits:]
                changed = True
            out.append(ins)
        if changed:
            bb.instructions = out


class _MinTailTileContext(tile.TileContext):
    """Tail: drain (split into 1-wait ops by the post-pass) + ONE barrier.
    Skips the semaphore clear + second barrier: this NEFF is executed
    once per compile, so sem state need not be restored."""

    def __init__(self, nc, minimal_tail=True, **kw):
        super().__init__(nc, **kw)
        self._minimal_tail = minimal_tail

    def _drain_and_barrier(self, tick_clock, wait_clock):
        if not self._minimal_tail:
            return super()._drain_and_barrier(tick_clock, wait_clock)
        drain_inst = self.nc.sync.drain()
        wait_clock.add_sem_waits(
            drain_inst.ins, ScopedClock({None: tick_clock.global_clock})
        )
        self.nc.all_engine_barrier()
        assert self.sems is not None
        popped = self.nc._tile_sem_poison_stack.pop()
        assert popped is self._sem_poison


def build_nc(use_f32r=True, minimal_tail=True):
    MDT = F32R if use_f32r else F32
    nc = bass.Bass()
    xt = nc.declare_dram_parameter("xt", [BPC, D, S], MDT, isOutput=False)
    w_in = nc.declare_dram_parameter("w_in", [D, H], MDT, isOutput=False)
    b_in = nc.declare_dram_parameter("b_in", [H, 1], F32, isOutput=False)
    wc = nc.declare_dram_parameter("wc", [H, DOUT], MDT, isOutput=False)
    y = nc.declare_dram_parameter("y", [BPC, 2, NG, G], F32, isOutput=True)

    rings = [nc.sync, nc.scalar]

    with _MinTailTileContext(nc, minimal_tail=minimal_tail) as tc:
        with (
            tc.tile_pool(name="consts", bufs=1) as consts,
            tc.tile_pool(name="xpool", bufs=2) as xpool,
            tc.tile_pool(name="hpool", bufs=2) as hpool,
            tc.tile_pool(name="wpool", bufs=2) as wpool,
            tc.tile_pool(name="opool", bufs=2) as opool,
            tc.tile_pool(name="psh", bufs=4, space="PSUM") as psh_pool,
            tc.tile_pool(name="psy", bufs=4, space="PSUM") as psy_pool,
        ):
            # PE warm-up: HAM un-throttles after ~3.4us of sustained PE
            # activity; burn that window on dummy matmuls while DMAs run.
            wz = consts.tile([128, 128], F32, tag="wz")
            nc.vector.memset(wz[:], 0.0)
            pwarm = psy_pool.tile([128, 128], F32, tag="psy", name="pwarm")
            for i in range(N_WARM):
                nc.tensor.matmul(
                    pwarm[:], wz[:], wz[:], start=(i == 0), stop=(i == N_WARM - 1)
                )

            # weights / bias first: small, so they clear the rings early
            wt = consts.tile([128, 2, H], MDT, tag="wt")       # [k, dc, h]
            nc.sync.dma_start(wt[:], w_in.rearrange("(dc k) h -> k dc h", dc=2))
            bt = consts.tile([128, 2], F32, tag="bt")          # [k, hc]
            nc.scalar.dma_start(bt[:], b_in.rearrange("(hc k) o -> k (hc o)", hc=2))
            wct = consts.tile([128, 2, DOUT], MDT, tag="wct")  # [k, hc, c]
            nc.scalar.dma_start(wct[:], wc.rearrange("(hc k) c -> k hc c", hc=2))

            # x loads: [128, 2(dc), TB] per (b, tb) in consumption order.
            # One shared tag with few bufs CASCADES the loads: concurrent
            # DMAs fair-share HBM, so capping in-flight transfers makes
            # early tiles complete early and PE can chase the stream.
            # (An explicit DMA->DMA dep chain measured the same mean but
            # intermittently wedged the device; slot-recycling is stable.)
            xb = {}
            for b in range(BPC):
                for tb in range(NTB):
                    t0 = tb * TB
                    xb[b, tb] = xpool.tile(
                        [128, 2, TB], MDT, tag="x", bufs=4, name=f"xb{b}{tb}"
                    )
                    rings[b].dma_start(
                        xb[b, tb][:],
                        xt[b, :, t0 : t0 + TB].rearrange(
                            "(dc p) t -> p dc t", p=128
                        ),
                    )

            for b in range(BPC):
                # relu(h)^T (batch edges handled by clipped stage-2 ranges)
                rh = {}
                for hc in range(2):
                    rh[hc] = hpool.tile(
                        [128, S], MDT, tag=f"rh{hc}", name=f"rh{hc}"
                    )

                # stage 1, latency order: tb outer so compute chases DMAs
                for tb in range(NTB):
                    for hc in range(2):
                        psh = psh_pool.tile(
                            [128, TB], F32, tag="psh", name=f"psh{hc}{tb}"
                        )
                        for dc in range(2):
                            nc.tensor.matmul(
                                psh[:],
                                wt[:, dc, hc * 128 : (hc + 1) * 128],
                                xb[b, tb][:, dc, :],
                                start=(dc == 0),
                                stop=(dc == 1),
                            )
                        nc.scalar.activation(
                            rh[hc][:, tb * TB : (tb + 1) * TB],
                            psh[:],
                            mybir.ActivationFunctionType.Relu,
                            bias=bt[:, hc : hc + 1],
                        )

                # stage 2: 4 groups of 512 tokens, each an exact-bank
                # [2, 512] PSUM tile at partition 0 (f32r), uniform N=512.
                psy = []
                for q in range(NG):
                    p = psy_pool.tile([2, G], F32, tag="psy", name=f"psy{q}")
                    for hc in range(2):
                        nc.tensor.matmul(
                            p[:],
                            wct[:, hc, :],
                            rh[hc][:, q * G : (q + 1) * G],
                            start=(hc == 0),
                            stop=(hc == 1),
                        )
                    psy.append(p)

                # pack all 4 groups into [98, 516] at 32-aligned partitions
                # with 2-token halos stitched from neighbor groups (zeros at
                # batch edges). For the LAST batch (kernel tail) split the
                # main copies across ACT and DVE — ACT is done with relu by
                # then, so the pack fills in parallel.
                pk = wpool.tile([98, G + 2 * PAD], F32, tag="pk")
                nc.vector.memset(pk[0:2, 0:PAD], 0.0)
                nc.vector.memset(pk[96:98, G + PAD :], 0.0)
                for q in range(NG):
                    r = pk[32 * q : 32 * q + 2, :]
                    if b == BPC - 1 and q % 2 == 1:
                        nc.scalar.activation(
                            r[:, PAD : G + PAD],
                            psy[q][:],
                            mybir.ActivationFunctionType.Copy,
                        )
                    else:
                        nc.vector.tensor_copy(r[:, PAD : G + PAD], psy[q][:])
                    if q > 0:
                        nc.vector.tensor_copy(
                            r[:, 0:PAD], psy[q - 1][:, G - PAD : G]
                        )
                    if q < NG - 1:
                        nc.vector.tensor_copy(
                            r[:, G + PAD :], psy[q + 1][:, 0:PAD]
                        )

                # 5-wide window sum in two free-dim halves so the first
                # half's output DMA overlaps the second half's adds.
                t1 = wpool.tile([98, G], F32, tag="t1")
                t2 = wpool.tile([98, G], F32, tag="t2")
                gy = wpool.tile([98, G], F32, tag="gy")
                HH = G // 2
                for h in range(2):
                    s = h * HH
                    nc.vector.tensor_add(
                        t1[:, s : s + HH], pk[:, s : s + HH], pk[:, s + 1 : s + 1 + HH]
                    )
                    nc.vector.tensor_add(
                        t2[:, s : s + HH], pk[:, s + 2 : s + 2 + HH], pk[:, s + 3 : s + 3 + HH]
                    )
                    nc.vector.tensor_add(
                        t1[:, s : s + HH], t1[:, s : s + HH], t2[:, s : s + HH]
                    )
                    nc.vector.tensor_add(
                        gy[:, s : s + HH], t1[:, s : s + HH], pk[:, s + 4 : s + 4 + HH]
                    )
                    # partition-strided output DMAs: c -> [4(q), HH]
                    for c in range(2):
                        rings[c].dma_start(
                            y[b, c, :, s : s + HH],
                            gy[c : c + 97 : 32, s : s + HH],
                        )

    _split_multi_waits(nc)
    return nc


_NC_CACHE = {}


def _get_nc(use_f32r=True, minimal_tail=True):
    key = (use_f32r, minimal_tail)
    if key not in _NC_CACHE:
        _NC_CACHE[key] = build_nc(use_f32r, minimal_tail)
    return _NC_CACHE[key]


def _run(x, W_in, b_in, W_v, W_out, b_out, use_f32r=True, minimal_tail=False,
         **spmd_kwargs):
    x = np.asarray(x, dtype=np.float32)
    W_in = np.ascontiguousarray(np.asarray(W_in, dtype=np.float32))
    b_in_col = np.ascontiguousarray(
        np.asarray(b_in, dtype=np.float32).reshape(H, 1)
    )
    Wc = (
        np.asarray(W_v, dtype=np.float64) @ np.asarray(W_out, dtype=np.float64)
    ).astype(np.float32)
    Wc = np.ascontiguousarray(Wc)

    # [B, S, D] -> [B, D, S] contiguous
    xT = np.ascontiguousarray(np.swapaxes(x, 1, 2))

    in_maps = [
        {
            "xt": xT[c * BPC : (c + 1) * BPC],
            "w_in": W_in,
            "b_in": b_in_col,
            "wc": Wc,
        }
        for c in range(NCORES)
    ]
    nc = _get_nc(use_f32r, minimal_tail)
    res = run_bass_kernel_spmd(
        nc, in_maps, core_ids=list(range(NCORES)), **spmd_kwargs
    )

    out = np.empty((B, S, DOUT), dtype=np.float32)
    for c in range(NCORES):
        yc = res.results[c]["y"]  # [BPC, 2(c), NG(q), G]
        for b in range(BPC):
            arr = yc[b].transpose(1, 2, 0).reshape(S, DOUT)  # t = q*G + i
            out[c * BPC + b] = arr
    out += np.asarray(b_out, dtype=np.float32).reshape(1, 1, DOUT)

    atten = np.ones((B, S, 1, 2 * PAD + 1), dtype=np.float32)
    return (out, atten), res


def kernel(x, W_in, b_in, W_q, W_k, W_v, W_out, b_out):
    (out, atten), _ = _run(x, W_in, b_in, W_v, W_out, b_out)
    return out, atten
